# revision 3
# baseline (speedup 1.0000x reference)
"""ContrastiveLoss on 8 Trainium2 NeuronCores.

loss = mean over ordered pairs i!=j of
    same(i,j)*(1-s_ij)^2 + diff(i,j)*relu(s_ij-0.5)^2,   s = cosine sim.

Decomposition (diagonal terms cancel up to (1-s_ii)^2 ~ 1e-6, negligible):
    T = sum_all relu2(s-0.5) + sum_same [(1-s)^2 - relu2(s-0.5)]
Rows are sorted by label on the host, so same-label pairs live in a +-127-row
band around the diagonal. Each core computes a 1024-row slab of the full
[N,N] relu2 sum (data-parallel, bf16 matmul against an on-chip transposed
copy of all embeddings) plus the masked band correction for classes that
start in its slab (row window extended 128 rows; non-owned rows get unique
sentinel labels so the is_equal mask excludes them).
"""
import json
import numpy as np

import concourse.bass as bass
import concourse.mybir as mybir
from concourse.masks import make_identity
from concourse.tile import TileContext
from concourse.vector_clock import ScopedClock, VectorClock

F32, BF16 = mybir.dt.float32, mybir.dt.bfloat16
A = mybir.AluOpType
AF = mybir.ActivationFunctionType

N, D, NCORES = 8192, 512, 8
ROWS = N // NCORES          # 1024 rows per core
EXT = 128                   # band extension past the slab
W = ROWS + EXT              # 1152 rows in the extended window
KT = D // 128               # 4 contraction tiles
CHUNK = 1024                # psum group width (2 banks)
MT = ROWS // 128            # 8 m-tiles per slab
NB = N // CHUNK             # 4 column groups
BT = W // 128               # 9 band row-tiles
BANDW = 384                 # band window width
MAIN_COLS = MT * NB         # 32 accumulator columns for the main term
OUT_COLS = 96               # main 0..63, corrA 64..72, corrB 73..81


# ---- walrus workarounds: <=1 sync wait per instruction ---------------------

class SplitWaitTileContext(TileContext):
    def _drain_and_barrier(self, tick_clock, wait_clock):
        gc = tick_clock.global_clock
        for p in range(27):
            v = gc[p]
            if v > 0:
                part = VectorClock()
                part.require_at_least(p, v)
                nop_inst = self.nc.sync.nop(nofuse=True, hint=f"tail_wait_{p}")
                wait_clock.add_sem_waits(nop_inst.ins, ScopedClock({None: part}))
        self.nc.sync.drain()
        self.nc.all_engine_barrier()
        assert self.sems is not None
        popped = self.nc._tile_sem_poison_stack.pop()
        assert popped is self._sem_poison
        self.nc.clear_and_free_semaphores(list(self.sems.allocated().values()))
        self.nc.all_engine_barrier()


def _split_waits_json(bir_json):
    d = json.loads(bir_json.decode() if isinstance(bir_json, bytes) else bir_json)
    ctr = 0
    for fn in d.get("functions", []):
        for blk in fn.get("blocks", []):
            out = []
            changed = False
            for inst in blk.get("instructions", []):
                si = inst.get("sync_info")
                waits = si.get("on_wait", []) if si else []
                if waits and len(waits) > 1:
                    si["on_wait"] = waits[-1:]
                    for wt in waits[:-1]:
                        ctr += 1
                        out.append({
                            "debug": inst.get("debug", 0),
                            "engine": inst["engine"],
                            "ins": [],
                            "name": f"I-wsplit-{ctr}",
                            "opcode": "NoOp",
                            "outs": [],
                            "sync_info": {"on_update": [], "on_wait": [wt]},
                        })
                    changed = True
                out.append(inst)
            if changed:
                blk["instructions"] = out
    return json.dumps(d).encode()


_waitfix_done = False


def _install_waitfix():
    global _waitfix_done
    if _waitfix_done:
        return
    _waitfix_done = True
    import concourse.bass_utils as bu
    orig = bu.compile_bir_kernel

    def fixed_compile(bir_json, tmpdir, neff_name="file.neff"):
        return orig(_split_waits_json(bir_json), tmpdir, neff_name)

    bu.compile_bir_kernel = fixed_compile
    try:
        import concourse.bass2jax as b2j
        b2j.compile_bir_kernel = fixed_compile
    except Exception:
        pass


# ---- device program --------------------------------------------------------

def _build_nc():
    nc = bass.Bass()

    def reg_const(dtype, value):
        t = nc.alloc_sbuf_tensor(f"const-{dtype.name}-{value}", [128, 1], dtype)
        nc.gpsimd.memset(t.ap(), value)
        nc.const_aps.aps[(dtype, value)] = t.ap()

    reg_const(F32, -0.5)
    nc.all_engine_barrier()

    emb_all = nc.dram_tensor("emb_all", [N, D], F32, kind="ExternalInput")
    emb_rows = nc.dram_tensor("emb_rows", [W, D], F32, kind="ExternalInput")
    labs_row = nc.dram_tensor("labs_row", [W, 1], F32, kind="ExternalInput")
    labs_col = nc.dram_tensor("labs_col", [128, W], F32, kind="ExternalInput")
    out = nc.dram_tensor("out", [128, OUT_COLS], F32, kind="ExternalOutput")

    ones_row = nc.const_aps.tensor(1.0, (128, D), F32)

    with SplitWaitTileContext(nc) as tc:
        with (
            tc.tile_pool(name="pers", bufs=1) as pers,
            tc.tile_pool(name="xin", bufs=3) as xin,
            tc.tile_pool(name="sqp", bufs=2) as sqp,
            tc.tile_pool(name="ssp", bufs=4) as ssp,
            tc.tile_pool(name="xnp", bufs=3) as xnp,
            tc.tile_pool(name="rp", bufs=3) as rp,
            tc.tile_pool(name="bandp", bufs=2) as bandp,
            tc.tile_pool(name="tpsum", bufs=2, space="PSUM") as tpsum,
            tc.tile_pool(name="mpsum", bufs=2, space="PSUM") as mpsum,
        ):
            ident = pers.tile([128, 128], F32)
            make_identity(nc, ident)

            ET = pers.tile([128, KT, N], BF16)
            ETr = pers.tile([128, KT, W], BF16)
            labc = pers.tile([128, W], F32)
            nc.sync.dma_start(labc, labs_col[:, :])
            acc = pers.tile([128, OUT_COLS], F32)
            nc.vector.memset(acc, 0.0)

            def prep(src, row0, dst, col0):
                # normalize 128 rows of src, transpose to dst[:, :, col0:+128]
                x = xin.tile([128, D], F32)
                nc.sync.dma_start(x, src[row0:row0 + 128, :])
                sqt = sqp.tile([128, D], F32)
                ss = ssp.tile([128, 1], F32)
                nc.scalar.activation(sqt, x, AF.Square, accum_out=ss)
                nrm = ssp.tile([128, 1], F32)
                nc.scalar.activation(nrm, ss, AF.Sqrt)
                inv = ssp.tile([128, 1], F32)
                nc.vector.reciprocal(inv, nrm)
                xn = xnp.tile([128, D], F32)
                nc.vector.scalar_tensor_tensor(
                    out=xn, in0=x, scalar=inv, in1=ones_row,
                    op0=A.mult, op1=A.mult)
                pt = tpsum.tile([128, KT, 128], F32)
                for k in range(KT):
                    nc.tensor.transpose(pt[:, k], xn[:, bass.ts(k, 128)], ident)
                nc.vector.tensor_copy(dst[:, :, col0:col0 + 128], pt[:, :, :])

            for rt in range(N // 128):
                prep(emb_all, rt * 128, ET, rt * 128)
            for rt in range(BT):
                prep(emb_rows, rt * 128, ETr, rt * 128)

            # main slab: rows = this core's 1024, cols = all 8192
            for m in range(MT):
                for nb in range(NB):
                    ps = mpsum.tile([128, CHUNK], F32)
                    for j in range(CHUNK // 512):
                        for k in range(KT):
                            nc.tensor.matmul(
                                ps[:, bass.ds(j * 512, 512)],
                                ETr[:, k, bass.ts(m, 128)],
                                ET[:, k, bass.ds(nb * CHUNK + j * 512, 512)],
                                start=(k == 0), stop=(k == KT - 1))
                    r = rp.tile([128, CHUNK], BF16)
                    nc.scalar.activation(r, ps, AF.Relu, bias=-0.5, scale=1.0)
                    junk = rp.tile([128, CHUNK], BF16, tag="junk")
                    nc.vector.scalar_tensor_tensor(
                        out=junk, in0=r, scalar=1.0, in1=r,
                        op0=A.mult, op1=A.mult,
                        accum_out=acc[:, m * NB + nb: m * NB + nb + 1])

            # band correction: masked sums over the extended window
            for ti in range(BT):
                win = min(max(0, (ti - 1) * 128), W - BANDW)
                bps = mpsum.tile([128, BANDW], F32, tag="bandps")
                for k in range(KT):
                    nc.tensor.matmul(
                        bps, ETr[:, k, bass.ts(ti, 128)],
                        ETr[:, k, bass.ds(win, BANDW)],
                        start=(k == 0), stop=(k == KT - 1))
                a_sb = bandp.tile([128, BANDW], F32, tag="a")
                nc.scalar.activation(a_sb, bps, AF.Square, bias=1.0, scale=-1.0)
                r_sb = bandp.tile([128, BANDW], F32, tag="r")
                nc.scalar.activation(r_sb, bps, AF.Relu, bias=-0.5, scale=1.0)
                lr = ssp.tile([128, 1], F32, tag="lr")
                nc.sync.dma_start(lr, labs_row[ti * 128:(ti + 1) * 128, :])
                j1 = bandp.tile([128, BANDW], F32, tag="j1")
                nc.vector.scalar_tensor_tensor(
                    out=j1, in0=labc[:, bass.ds(win, BANDW)], scalar=lr,
                    in1=a_sb, op0=A.is_equal, op1=A.mult,
                    accum_out=acc[:, MAIN_COLS + ti: MAIN_COLS + ti + 1])
                rm = bandp.tile([128, BANDW], F32, tag="rm")
                nc.vector.scalar_tensor_tensor(
                    out=rm, in0=labc[:, bass.ds(win, BANDW)], scalar=lr,
                    in1=r_sb, op0=A.is_equal, op1=A.mult)
                j2 = bandp.tile([128, BANDW], F32, tag="j2")
                nc.vector.scalar_tensor_tensor(
                    out=j2, in0=rm, scalar=1.0, in1=rm,
                    op0=A.mult, op1=A.mult,
                    accum_out=acc[:, MAIN_COLS + BT + ti: MAIN_COLS + BT + ti + 1])

            nc.sync.dma_start(out[:, :], acc)
    return nc


_NC_CACHE = {}


def _get_nc():
    if "nc" not in _NC_CACHE:
        _install_waitfix()
        _NC_CACHE["nc"] = _build_nc()
    return _NC_CACHE["nc"]


# ---- host wrapper ----------------------------------------------------------

def kernel(embeddings, labels):
    from concourse.bass_utils import run_bass_kernel_spmd

    emb = np.asarray(embeddings, dtype=np.float32)
    lab = np.asarray(labels)
    perm = np.argsort(lab, kind="stable")
    E_s = np.ascontiguousarray(emb[perm])
    labs = lab[perm].astype(np.int64)

    # class start index per sorted row
    change = np.empty(N, dtype=bool)
    change[0] = True
    change[1:] = labs[1:] != labs[:-1]
    start_idx = np.maximum.accumulate(np.where(change, np.arange(N), -1))
    counts = np.bincount(labs.astype(np.int64))
    assert counts.max() <= EXT, "class larger than band extension"

    in_maps = []
    for c in range(NCORES):
        lo = c * ROWS
        hi = lo + W
        rows = np.ones((W, D), dtype=np.float32)  # dummy norm-1-safe pad
        take = min(hi, N) - lo
        rows[:take] = E_s[lo:lo + take]
        owned = np.zeros(W, dtype=bool)
        owned[:take] = (start_idx[lo:lo + take] >= lo) & \
                       (start_idx[lo:lo + take] < lo + ROWS)
        lab_ext = np.full(W, -1.0, dtype=np.float64)
        lab_ext[:take] = labs[lo:lo + take]
        lrow = np.where(owned, lab_ext, -1000.0 - np.arange(W))
        lcol = np.where(owned, lab_ext, -2000.0 - np.arange(W))
        in_maps.append({
            "emb_all": E_s,
            "emb_rows": rows,
            "labs_row": lrow.astype(np.float32)[:, None].copy(),
            "labs_col": np.broadcast_to(
                lcol.astype(np.float32)[None, :], (128, W)).copy(),
        })

    nc = _get_nc()
    res = run_bass_kernel_spmd(nc, in_maps, core_ids=list(range(NCORES)))
    kernel._last_result = res

    total = 0.0
    for c in range(NCORES):
        o = res.results[c]["out"].astype(np.float64)
        main = o[:, 0:MAIN_COLS].sum()
        corr_a = o[:, MAIN_COLS:MAIN_COLS + BT].sum()
        corr_b = o[:, MAIN_COLS + BT:MAIN_COLS + 2 * BT].sum()
        total += main + corr_a - corr_b
    return np.float32(total / (N * (N - 1)))
